# revision 1
# baseline (speedup 1.0000x reference)
"""Bidirectional LSTM (L=512, B=64, E=512, H=512 per dir) on 8 NeuronCores.

Strategy (SPMD, zero cross-core communication):
  - Batch-parallel over B: core c owns samples [8c, 8c+8), both directions.
  - Phase 1: embedding gather (indirect DMA) -> X; X.T via PE transposes;
    g_pre = X @ Wih.T + (b_ih + b_hh) with big matmuls; stored to a DRAM
    scratch in source-time order for both directions.
  - Phase 2: 512 fully-unrolled recurrence steps. Per step, gates
    g = g_pre[t] + h @ Whh.T accumulate in PSUM: h-part via 4 K-chunk
    matmuls, g_pre injected through the PE with an eye(16) stationary
    (DMA cannot touch PSUM). The four (direction, h-half) units map to the
    four 32-column groups of the PE array / PSUM partition blocks
    (base partitions 0/32/64/96, 8 rows each) so their matmuls execute
    concurrently; emission is wave-interleaved across groups.
  - Gate columns are host-permuted to [i|f|o|g] per 256-wide h-half so one
    sigmoid op covers i,f,o and one tanh covers g.
  - Padding mask folded into the sigmoid bias (per-partition bias AP):
    sigma(x - 1e9*(1-m)) == 0 at padded steps => c_t = h_t = 0 exactly as
    the reference's post-step h*m, c*m masking (mask is monotone).
  - h.T for the next step via PE transposes (cols land at the partition
    block offsets, directly usable as the next matmul's stationary).
"""

import os
import sys

sys.path.insert(0, "/opt/trn_rl_repo")

import numpy as np

L, B, E, V = 512, 64, 512, 32000
H = 512           # hidden per direction
NB = 8            # batch per core
NCORES = 8
HH = 256          # h per half
GW = 1024         # gate cols per half (4 gates x 256)

_BUILT = {}


def _split_sync_waits(nc, max_waits=1):
    """This container's walrus rejects >1 sync-wait per instruction
    (CoreV3GenImpl setupSyncWait). Split extras onto preceding same-engine
    NoOps."""
    import concourse.mybir as mybir

    ctr = 0
    for fn in nc.m.functions:
        for blk in fn.blocks:
            out = []
            changed = False
            for inst in blk.instructions:
                si = inst.sync_info
                if si is not None and si.on_wait and len(si.on_wait) > max_waits:
                    waits = list(si.on_wait)
                    extra, keep = waits[:-max_waits], waits[-max_waits:]
                    for i in range(0, len(extra), max_waits):
                        ctr += 1
                        nop = mybir.InstNoOp(
                            name=f"bass_waitsplit_{ctr}", ins=[], outs=[])
                        nop.engine = inst.engine
                        nop.sync_info = mybir.SyncInfo(
                            on_wait=extra[i:i + max_waits], on_update=[])
                        out.append(nop)
                    si.on_wait = keep
                    changed = True
                out.append(inst)
            if changed:
                blk.instructions[:] = out


def _gate_perm():
    """New gate-column order (length 4H): per half q in {0,1}:
    [i[256q:256q+256], f[...], o[...], g[...]] referencing original rows
    i=0:512, f=512:1024, g=1024:1536, o=1536:2048."""
    p = []
    for q in range(2):
        s = 256 * q
        p += list(range(s, s + 256))            # i
        p += list(range(512 + s, 512 + s + 256))   # f
        p += list(range(1536 + s, 1536 + s + 256))  # o
        p += list(range(1024 + s, 1024 + s + 256))  # g
    return np.array(p, dtype=np.int64)


def _build(nsteps=L, ntiles=L * NB // 128):
    key = (nsteps, ntiles)
    if key in _BUILT:
        return _BUILT[key]
    import concourse.bass as bass
    import concourse.mybir as mybir
    import concourse.tile as tile
    from concourse.masks import make_identity

    f32 = mybir.dt.float32
    nrows = ntiles * 128

    nc = bass.Bass()
    emb = nc.dram_tensor("emb", [V, E], f32, kind="ExternalInput")
    toks = nc.dram_tensor("toks", [128, ntiles], mybir.dt.int32,
                          kind="ExternalInput")
    tokmask = nc.dram_tensor("tokmask", [128, ntiles], f32,
                             kind="ExternalInput")
    sigbias = nc.dram_tensor("sigbias", [128, nsteps], f32, kind="ExternalInput")
    wihT_d = nc.dram_tensor("wihT", [2, 4, 128, 2048], f32, kind="ExternalInput")
    whhT_d = nc.dram_tensor("whhT", [2, 4, 128, 2048], f32, kind="ExternalInput")
    gbias_d = nc.dram_tensor("gbias", [2, 2048], f32, kind="ExternalInput")
    out_d = nc.dram_tensor("out", [nsteps, NB, 2 * H], f32, kind="ExternalOutput")

    with tile.TileContext(nc) as tc:
        with (
            tc.tile_pool(name="persist", bufs=1) as pp,
            tc.tile_pool(name="dram", bufs=1, space="DRAM") as dp,
        ):
            # ---- persistent SBUF ----
            wihT = pp.tile([128, 2, 4, 2048], f32)
            whhT = pp.tile([128, 2, 4, 2048], f32)
            for d in range(2):
                for k in range(4):
                    nc.sync.dma_start(wihT[:, d, k, :], wihT_d[d, k])
                    nc.sync.dma_start(whhT[:, d, k, :], whhT_d[d, k])
            gbias = pp.tile([1, 2, 2048], f32)
            nc.sync.dma_start(gbias[:, 0, :], gbias_d[0:1, :])
            nc.sync.dma_start(gbias[:, 1, :], gbias_d[1:2, :])
            sb = pp.tile([128, nsteps], f32)
            nc.sync.dma_start(sb[:], sigbias[:])
            ident = pp.tile([128, 128], f32)
            make_identity(nc, ident[:])
            ones1 = pp.tile([1, 128], f32)
            nc.vector.memset(ones1[:], 1.0)
            toks_t = pp.tile([128, ntiles], mybir.dt.int32)
            nc.sync.dma_start(toks_t[:], toks[:])
            tmask_t = pp.tile([128, ntiles], f32)
            nc.sync.dma_start(tmask_t[:], tokmask[:])

            gpre = dp.tile([nsteps, 16, 2048], f32)

            # ================= Phase 1: g_pre =================
            with (
                tc.tile_pool(name="p1", bufs=3) as p1,
                tc.tile_pool(name="p1ps", bufs=2, space="PSUM") as p1ps,
                tc.tile_pool(name="p1tr", bufs=2, space="PSUM") as p1tr,
            ):
                for r in range(ntiles):
                    xt = p1.tile([128, E], f32)
                    nc.gpsimd.indirect_dma_start(
                        out=xt[:], out_offset=None, in_=emb[:],
                        in_offset=bass.IndirectOffsetOnAxis(
                            ap=toks_t[:, r:r + 1], axis=0))
                    nc.vector.tensor_scalar_mul(xt[:], xt[:], tmask_t[:, r:r + 1])
                    xT = p1.tile([128, 4, 128], f32)
                    for k in range(4):
                        trp = p1tr.tile([128, 128], f32, space="PSUM")
                        nc.tensor.transpose(
                            out=trp[:], in_=xt[:, 128 * k:128 * (k + 1)],
                            identity=ident[:])
                        if k % 2 == 0:
                            nc.vector.tensor_copy(xT[:, k, :], trp[:])
                        else:
                            nc.scalar.copy(xT[:, k, :], trp[:])
                    for d in range(2):
                        for nch in range(4):
                            n0 = 512 * nch
                            gps = p1ps.tile([128, 512], f32, space="PSUM")
                            for k in range(4):
                                nc.tensor.matmul(
                                    gps[:], xT[:, k, :],
                                    wihT[:, d, k, n0:n0 + 512],
                                    start=(k == 0), stop=False)
                            nc.tensor.matmul(
                                gps[:], ones1[:], gbias[:, d, n0:n0 + 512],
                                start=False, stop=True)
                            gsb = p1.tile([128, 512], f32)
                            if nch % 2 == 0:
                                nc.vector.tensor_copy(gsb[:], gps[:])
                            else:
                                nc.scalar.copy(gsb[:], gps[:])
                            # rows of this tile are (l = 16r + i, b); write to
                            # gpre[l, 8d + b, n0:n0+512]
                            nc.sync.dma_start(
                                gpre[16 * r:16 * (r + 1),
                                     8 * d:8 * d + 8, n0:n0 + 512],
                                gsb[:])

            # ================= Phase 2: recurrence =================
            with (
                tc.tile_pool(name="p2", bufs=2) as p2,
                tc.tile_pool(name="p2g", bufs=4) as p2g,
                tc.tile_pool(name="p2ps", bufs=2, space="PSUM") as p2ps,
                tc.tile_pool(name="p2tr", bufs=4, space="PSUM") as p2tr,
            ):
                hT_prev = None
                c_prev = None
                for t in range(nsteps):
                    gp = p2g.tile([16, 2048], f32)
                    nc.sync.dma_start(gp[0:8, :], gpre[t, 0:8, :])
                    nc.sync.dma_start(
                        gp[8:16, :], gpre[nsteps - 1 - t, 8:16, :])

                    gps = p2ps.tile([128, 1024], f32, space="PSUM")
                    # wave-interleaved matmuls across the 4 groups
                    # group g: direction d = g >> 1, half q = g & 1,
                    # psum partitions [32g, 32g+8)
                    # M=32 everywhere (cols 8:32 of each group compute garbage
                    # from uninitialized lanes; block-diagonal so it never
                    # touches the real 8 rows) so the whole PSUM tile is
                    # written and downstream full-width reads are clean.
                    for nb_ in range(2):
                        pcol = 512 * nb_
                        if hT_prev is not None:
                            for k in range(4):
                                for g in range(4):
                                    d, q = g >> 1, g & 1
                                    n0 = GW * q + pcol
                                    lcol = 32 * (k // 2) + 64 * d
                                    # per-group start: clears the 2KB zero
                                    # region within this group's partitions
                                    # only. skip_group_check silences the
                                    # sim's partition-blind group tracker.
                                    nc.tensor.matmul(
                                        gps[32 * g:32 * g + 32, pcol:pcol + 512],
                                        hT_prev[:, k, lcol:lcol + 32],
                                        whhT[:, d, k, n0:n0 + 512],
                                        start=(k == 0), stop=False,
                                        tile_position=(0, 32 * g),
                                        skip_group_check=True)
                        for g in range(4):
                            d, q = g >> 1, g & 1
                            n0 = GW * q + pcol
                            # eye cols 8d:8d+32: row j<8 picks gpre row 8d+j,
                            # rows 8..32 hit eye rows >=16 -> zero
                            nc.tensor.matmul(
                                gps[32 * g:32 * g + 32, pcol:pcol + 512],
                                ident[0:16, 8 * d:8 * d + 32],
                                gp[:, n0:n0 + 512],
                                start=(hT_prev is None), stop=True,
                                tile_position=(0, 32 * g),
                                skip_group_check=True)

                    sig = p2.tile([128, 768], f32)
                    nc.scalar.activation(
                        sig[:], gps[:, 0:768],
                        mybir.ActivationFunctionType.Sigmoid,
                        bias=sb[:, t:t + 1], scale=1.0)
                    tg = p2.tile([128, 256], f32)
                    nc.scalar.activation(
                        tg[:], gps[:, 768:1024],
                        mybir.ActivationFunctionType.Tanh)

                    c_new = p2.tile([128, 256], f32, tag="c_state")
                    if c_prev is None:
                        nc.vector.tensor_mul(c_new[:], sig[:, 0:256], tg[:])
                    else:
                        t1 = p2.tile([128, 256], f32)
                        nc.vector.tensor_mul(t1[:], sig[:, 0:256], tg[:])
                        t2 = p2.tile([128, 256], f32)
                        nc.vector.tensor_mul(t2[:], sig[:, 256:512], c_prev[:])
                        nc.vector.tensor_add(c_new[:], t1[:], t2[:])
                    tc_ = p2.tile([128, 256], f32)
                    nc.scalar.activation(
                        tc_[:], c_new[:], mybir.ActivationFunctionType.Tanh)
                    h = p2.tile([128, 256], f32)
                    nc.vector.tensor_mul(h[:], sig[:, 512:768], tc_[:])

                    hT = p2.tile([128, 4, 128], f32, tag="hT_state")
                    for k in range(4):
                        off = 128 * (k % 2)
                        trp = p2tr.tile([128, 128], f32, space="PSUM")
                        nc.tensor.transpose(
                            out=trp[:], in_=h[:, off:off + 128],
                            identity=ident[:])
                        if k % 2 == 0:
                            nc.vector.tensor_copy(hT[:, k, :], trp[:])
                        else:
                            nc.scalar.copy(hT[:, k, :], trp[:])

                    # output: fwd -> out[t, :, 0:512]; bwd -> out[L-1-t, :, 512:1024]
                    nc.sync.dma_start(out_d[t, :, 0:256], h[0:8, :])
                    nc.sync.dma_start(out_d[t, :, 256:512], h[32:40, :])
                    nc.sync.dma_start(
                        out_d[nsteps - 1 - t, :, 512:768], h[64:72, :])
                    nc.sync.dma_start(
                        out_d[nsteps - 1 - t, :, 768:1024], h[96:104, :])

                    hT_prev = hT
                    c_prev = c_new

    _BUILT[key] = nc
    return nc


def _ensure_split(nc):
    if not getattr(nc, "_waitsplit_done", False):
        _split_sync_waits(nc)
        nc._waitsplit_done = True


def _prep_core_inputs(c, tokens, mask, emb_table, wihT, whhT, gbias, sigbias_all,
                      nsteps, ntiles):
    s = slice(NB * c, NB * (c + 1))
    # row r*128+p of the (l, b) flattening, laid out [partition, tile]
    toks_c = np.clip(tokens[:nsteps, s], 0, V - 1).astype(np.int32)
    toks_c = toks_c.reshape(ntiles, 128).T
    tmask_c = mask[:nsteps, s].astype(np.float32).reshape(ntiles, 128).T
    return {
        "emb": emb_table,
        "toks": np.ascontiguousarray(toks_c),
        "tokmask": np.ascontiguousarray(tmask_c),
        "sigbias": np.ascontiguousarray(sigbias_all[c]),
        "wihT": wihT,
        "whhT": whhT,
        "gbias": gbias,
    }


def kernel(tokens, mask, emb_table, W_ih_f, W_hh_f, b_ih_f, b_hh_f,
           W_ih_b, W_hh_b, b_ih_b, b_hh_b, _nsteps=L, _trace=False):
    from concourse.bass_utils import run_bass_kernel_spmd

    tokens = np.asarray(tokens)
    mask = np.asarray(mask, dtype=np.float32)
    emb_table = np.ascontiguousarray(np.asarray(emb_table, dtype=np.float32))

    perm = _gate_perm()
    wihT = np.stack([
        np.asarray(W_ih_f, np.float32)[perm].T.reshape(4, 128, 2048),
        np.asarray(W_ih_b, np.float32)[perm].T.reshape(4, 128, 2048),
    ]).copy()
    whhT = np.stack([
        np.asarray(W_hh_f, np.float32)[perm].T.reshape(4, 128, 2048),
        np.asarray(W_hh_b, np.float32)[perm].T.reshape(4, 128, 2048),
    ]).copy()
    gbias = np.stack([
        (np.asarray(b_ih_f, np.float32) + np.asarray(b_hh_f, np.float32))[perm],
        (np.asarray(b_ih_b, np.float32) + np.asarray(b_hh_b, np.float32))[perm],
    ]).copy()

    nsteps = _nsteps
    ntiles = nsteps * NB // 128

    # sigbias[core][p, t]: fwd blocks (p in [0,8) u [32,40)): -1e9*(1-mask[t, b]);
    # bwd blocks (p in [64,72) u [96,104)): -1e9*(1-mask[L-1-t, b])
    sigbias_all = np.zeros((NCORES, 128, nsteps), np.float32)
    for c in range(NCORES):
        mk = mask[:nsteps, NB * c:NB * (c + 1)]          # [T, 8]
        fwd = -1e9 * (1.0 - mk.T)                        # [8, T]
        bwd = -1e9 * (1.0 - mk[::-1].T)
        for base in (0, 32):
            sigbias_all[c, base:base + 8] = fwd
        for base in (64, 96):
            sigbias_all[c, base:base + 8] = bwd

    nc = _build(nsteps, ntiles)
    _ensure_split(nc)
    in_maps = [
        _prep_core_inputs(c, tokens, mask, emb_table, wihT, whhT, gbias,
                          sigbias_all, nsteps, ntiles)
        for c in range(NCORES)
    ]
    res = run_bass_kernel_spmd(nc, in_maps, core_ids=list(range(NCORES)),
                               trace=_trace)
    out = np.empty((nsteps, B, 2 * H), np.float32)
    for c in range(NCORES):
        out[:, NB * c:NB * (c + 1), :] = res.results[c]["out"]
    kernel._last_results = res
    return out



# revision 2
# speedup vs baseline: 1.0320x; 1.0320x over previous
"""Bidirectional LSTM on 8 NeuronCores — v7: per-gate-block PSUM tiles.

Structure (dir-sharding, flipped orientation, transpose-gather, -1e9 mask)
as v2/v4, but gpre never leaves PSUM:
  - The gates PSUM is organized in 8-step groups: one [128, 2048] f32 tile
    (4 banks) holds cols m*128 + 16*t8 + b for the group's 8 steps.
  - Phase-1 (Wih GEMM + mask + bias, all PE matmuls) writes each group's
    tile one group ahead of the recurrence; the per-step h-matmuls then
    accumulate on top (start=False) and the activations read the psum
    directly. No eye-inject, no psum->sbuf copies: Act/DVE run ONLY the
    recurrence chain, so the tile scheduler cannot wedge bulk work into
    the latency-critical path.
  - PSUM budget: 2 group tiles x 4 banks = all 8 banks.
"""

import sys

sys.path.insert(0, "/opt/trn_rl_repo")

import numpy as np
import ml_dtypes

L, B, E, V = 512, 64, 512, 32000
H = 512
NB = 16
NCORES = 8
GRP = 8            # steps per psum group
WIN = 32           # steps per output-ring window

_BUILT = {}


def _split_sync_waits(nc, max_waits=1):
    import concourse.mybir as mybir

    ctr = 0
    for fn in nc.m.functions:
        for blk in fn.blocks:
            out = []
            changed = False
            for inst in blk.instructions:
                si = inst.sync_info
                if si is not None and si.on_wait and len(si.on_wait) > max_waits:
                    waits = list(si.on_wait)
                    extra, keep = waits[:-max_waits], waits[-max_waits:]
                    for i in range(0, len(extra), max_waits):
                        ctr += 1
                        nop = mybir.InstNoOp(
                            name=f"bass_waitsplit_{ctr}", ins=[], outs=[])
                        nop.engine = inst.engine
                        nop.sync_info = mybir.SyncInfo(
                            on_wait=extra[i:i + max_waits], on_update=[])
                        out.append(nop)
                    si.on_wait = keep
                    changed = True
                out.append(inst)
            if changed:
                blk.instructions[:] = out


def _build(nsteps=L):
    key = nsteps
    if key in _BUILT:
        return _BUILT[key]
    import concourse.bass as bass
    import concourse.mybir as mybir
    import concourse.tile as tile
    from concourse.masks import make_identity

    f32 = mybir.dt.float32
    bf16 = mybir.dt.bfloat16
    i16 = mybir.dt.int16
    AF = mybir.ActivationFunctionType
    ngrp = nsteps // GRP
    NGB = GRP * NB           # idx count per group (128)

    nc = bass.Bass()
    emb = nc.dram_tensor("emb", [V, E], bf16, kind="ExternalInput")
    toks = nc.dram_tensor("toks", [128, nsteps // GRP], mybir.dt.int32,
                          kind="ExternalInput")
    maskbar_d = nc.dram_tensor("maskbar", [1, NB * nsteps], bf16,
                               kind="ExternalInput")
    wihT_d = nc.dram_tensor("wihT", [4, 128, 2048], bf16, kind="ExternalInput")
    whhT_d = nc.dram_tensor("whhT", [4, 128, 2048], bf16, kind="ExternalInput")
    gbiasT_d = nc.dram_tensor("gbiasT", [1, 2048], bf16, kind="ExternalInput")
    out_d = nc.dram_tensor("out", [128, nsteps, 4, NB], bf16,
                           kind="ExternalOutput")

    with tile.TileContext(nc) as tc:
        with (
            tc.tile_pool(name="persist", bufs=1) as pp,
            tc.tile_pool(name="xT", bufs=3) as xp,
            tc.tile_pool(name="xr", bufs=3) as xrp,
            tc.tile_pool(name="ring", bufs=2) as rp,
            tc.tile_pool(name="ew", bufs=3) as ep,
            tc.tile_pool(name="cst", bufs=2) as cp,
            tc.tile_pool(name="gps", bufs=2, space="PSUM") as gpsp,
        ):
            # ---- persistent SBUF ----
            wihT = pp.tile([128, 4, 2048], bf16)
            whhT = pp.tile([128, 4, 2048], bf16)
            for k in range(4):
                nc.sync.dma_start(wihT[:, k, :], wihT_d[k])
                nc.sync.dma_start(whhT[:, k, :], whhT_d[k])
            # bias as a K=1 stationary: gbiasT[0, j] = bias of gate j
            gbiasT = pp.tile([1, 2048], bf16)
            nc.sync.dma_start(gbiasT[:], gbiasT_d[:])
            toks_t = pp.tile([128, nsteps // GRP], mybir.dt.int32)
            nc.sync.dma_start(toks_t[:], toks[:])
            maskbar = pp.tile([1, NB * nsteps], bf16)
            nc.sync.dma_start(maskbar[:], maskbar_d[:])
            mneg = pp.tile([1, 128], bf16)
            nc.vector.memset(mneg[:], -1e9)
            ones1 = pp.tile([1, 128], bf16)
            nc.vector.memset(ones1[:], 1.0)
            identb = pp.tile([128, 128], bf16)
            make_identity(nc, identb[:])

            def gather(g, tiles):
                """Gather group g's embeddings and transpose to xT via the
                PE, borrowing the group's own (still idle) gate psum tiles
                as transpose scratch."""
                xr = xrp.tile([128, E], bf16)
                nc.gpsimd.indirect_dma_start(
                    out=xr[:], out_offset=None, in_=emb[:],
                    in_offset=bass.IndirectOffsetOnAxis(
                        ap=toks_t[:, g:g + 1], axis=0))
                xT = xp.tile([128, 4, NGB], bf16)
                for k in range(4):
                    scratch = tiles[k][:, 0:64].bitcast(bf16)
                    nc.tensor.transpose(out=scratch, in_=xr[:, 128 * k:
                                                            128 * (k + 1)],
                                        identity=identb[:])
                    if k % 2 == 0:
                        nc.vector.tensor_copy(xT[:, k, :], scratch)
                    else:
                        nc.scalar.copy(xT[:, k, :], scratch)
                return xT

            def p1_group(xT_g, tiles, g):
                """gpre for group g -> four per-gate psum tiles (1 bank
                each): tile[gb][col = 128*m' + 16*t8 + b]."""
                for gb in range(4):
                    ps = tiles[gb]
                    for mp in range(4):
                        m = 4 * gb + mp
                        o = 128 * mp
                        for k in range(4):
                            nc.tensor.matmul(
                                ps[:, o:o + 128],
                                wihT[:, k, 128 * m:128 * (m + 1)], xT_g[:, k, :],
                                start=(k == 0 and mp == 0), stop=False,
                                skip_group_check=True)
                        if gb < 2:
                            nc.tensor.matmul(
                                ps[:, o:o + 128], mneg[:, :],
                                maskbar[:, NGB * g:NGB * (g + 1)],
                                start=False, stop=False,
                                skip_group_check=True)
                        nc.tensor.matmul(
                            ps[:, o:o + 128],
                            gbiasT[:, 128 * m:128 * (m + 1)],
                            ones1[:, 0:NGB],
                            start=False, stop=True,
                            skip_group_check=True)

            def alloc_group():
                return [gpsp.tile([128, 512], f32, space="PSUM",
                                  name=f"gps_{nm}") for nm in "fiog"]

            # group 0 phase 1 up front
            gps_cur = alloc_group()
            xT0 = gather(0, gps_cur)
            p1_group(xT0, gps_cur, 0)
            gps_next = None

            ring_prev = None
            ring_cur = None
            c_prev = None

            for t in range(nsteps):
                g, t8 = t // GRP, t % GRP
                w, tp = t // WIN, t % WIN
                if t8 == 0 and g + 1 < ngrp:
                    with tc.high_priority(offset=-1000000):
                        gps_next = alloc_group()
                        xT_g = gather(g + 1, gps_next)
                        p1_group(xT_g, gps_next, g + 1)
                if tp == 0:
                    ring_prev, ring_cur = ring_cur, rp.tile(
                        [128, WIN, 4 * NB], bf16)

                # ---- recurrence matmuls: accumulate h @ WhhT into psum ----
                co = 16 * t8
                if t > 0:
                    hmov = (ring_prev[:, WIN - 1, :] if tp == 0
                            else ring_cur[:, tp - 1, :])
                    for m in range(16):
                        gb, mp = divmod(m, 4)
                        for k in range(4):
                            nc.tensor.matmul(
                                gps_cur[gb][:, 128 * mp + co:
                                            128 * mp + co + 16],
                                whhT[:, k, 128 * m:128 * (m + 1)],
                                hmov[:, NB * k:NB * (k + 1)],
                                start=False, stop=(k == 3),
                                skip_group_check=True)

                # ---- elementwise chain ----
                q = 4 * NB  # 64
                # per-gate psum tiles give each activation a precise PE
                # wait (its own block's last matmul); Act same-engine pins
                # are stripped post-build
                def gv(gb):
                    return gps_cur[gb][:].rearrange(
                        "p (m c) -> p m c", c=128)[:, :, co:co + 16]
                sigf = ep.tile([128, q], f32)
                nc.scalar.activation(sigf[:], gv(0), AF.Sigmoid)
                sigi = ep.tile([128, q], f32)
                nc.scalar.activation(sigi[:], gv(1), AF.Sigmoid)
                tg = ep.tile([128, q], f32)
                nc.scalar.activation(tg[:], gv(3), AF.Tanh)
                sigo = ep.tile([128, q], f32)
                nc.scalar.activation(sigo[:], gv(2), AF.Sigmoid)

                c_new = cp.tile([128, q], f32, tag="c_state")
                if c_prev is None:
                    nc.vector.tensor_mul(c_new[:], sigi[:], tg[:])
                else:
                    t2 = ep.tile([128, q], f32)
                    nc.vector.tensor_mul(t2[:], sigf[:], c_prev[:])
                    t1 = ep.tile([128, q], f32)
                    nc.vector.tensor_mul(t1[:], sigi[:], tg[:])
                    nc.vector.tensor_add(c_new[:], t1[:], t2[:])
                tc_ = ep.tile([128, q], f32)
                nc.scalar.activation(tc_[:], c_new[:], AF.Tanh)
                nc.vector.tensor_mul(ring_cur[:, tp, :], sigo[:], tc_[:])
                c_prev = c_new

                if tp == WIN - 1:
                    nc.sync.dma_start(
                        out_d[:, WIN * w:WIN * (w + 1)], ring_cur[:])
                if t8 == GRP - 1:
                    gps_cur = gps_next

    _BUILT[key] = nc
    return nc


def _strip_act_pins(nc):
    import concourse.mybir as mybir
    n = 0
    for fn in nc.m.functions:
        for blk in fn.blocks:
            for inst in blk.instructions:
                if type(inst).__name__ != 'InstActivation':
                    continue
                si = inst.sync_info
                if si is None or not si.on_wait:
                    continue
                ins0 = inst.ins[0]
                mref = getattr(ins0, 'memref', '')
                if not (isinstance(mref, str) and mref.startswith('gps')):
                    continue
                keep = [w for w in si.on_wait
                        if not getattr(w, 'ant_name', '')
                        .startswith('Activation')]
                if len(keep) != len(si.on_wait):
                    si.on_wait = keep
                    n += 1
    return n


NOSPLIT = False


def _ensure_split(nc):
    if NOSPLIT:
        return
    if not getattr(nc, "_waitsplit_done", False):
        _strip_act_pins(nc)
        _split_sync_waits(nc)
        nc._waitsplit_done = True


# gate row order [f | i | o | g] (reference order is i, f, g, o)
_PERM = np.concatenate([
    np.arange(512, 1024), np.arange(0, 512),
    np.arange(1536, 2048), np.arange(1024, 1536)])


def _prep_weights(W_ih, W_hh, b_ih, b_hh):
    wihT = np.ascontiguousarray(
        np.asarray(W_ih, np.float32)[_PERM].T.reshape(4, 128, 2048)
    ).astype(ml_dtypes.bfloat16)
    whhT = np.ascontiguousarray(
        np.asarray(W_hh, np.float32)[_PERM].T.reshape(4, 128, 2048)
    ).astype(ml_dtypes.bfloat16)
    gbiasT = np.ascontiguousarray(
        (np.asarray(b_ih, np.float32)
         + np.asarray(b_hh, np.float32))[_PERM].reshape(1, 2048)
    ).astype(ml_dtypes.bfloat16)
    return wihT, whhT, gbiasT


def kernel(tokens, mask, emb_table, W_ih_f, W_hh_f, b_ih_f, b_hh_f,
           W_ih_b, W_hh_b, b_ih_b, b_hh_b, _nsteps=L, _trace=False):
    from concourse.bass_utils import run_bass_kernel_spmd

    tokens = np.asarray(tokens)
    mask = np.asarray(mask, dtype=np.float32)
    emb_bf = np.ascontiguousarray(
        np.asarray(emb_table, dtype=np.float32)).astype(ml_dtypes.bfloat16)

    wf = _prep_weights(W_ih_f, W_hh_f, b_ih_f, b_hh_f)
    wb = _prep_weights(W_ih_b, W_hh_b, b_ih_b, b_hh_b)

    nsteps = _nsteps
    nc = _build(nsteps)
    _ensure_split(nc)

    in_maps = []
    for c in range(NCORES):
        d, qtr = divmod(c, 4)
        s = slice(NB * qtr, NB * (qtr + 1))
        tok_c = tokens[:nsteps, s]
        mask_c = mask[:nsteps, s]
        if d == 1:
            tok_c, mask_c = tok_c[::-1], mask_c[::-1]
        tc_clip = np.clip(tok_c, 0, V - 1).astype(np.int32)
        # row p of gather group g holds token (t = GRP*g + p//16, b = p%16)
        toks_sb = np.ascontiguousarray(
            tc_clip.reshape(nsteps // GRP, GRP * NB).T)
        maskbar = np.ascontiguousarray(
            (1.0 - mask_c).reshape(1, NB * nsteps)).astype(ml_dtypes.bfloat16)
        wihT, whhT, gbiasT = wf if d == 0 else wb
        in_maps.append({
            "emb": emb_bf,
            "toks": toks_sb,
            "maskbar": maskbar,
            "wihT": wihT,
            "whhT": whhT,
            "gbiasT": gbiasT,
        })

    res = run_bass_kernel_spmd(nc, in_maps, core_ids=list(range(NCORES)),
                               trace=_trace)
    out = np.empty((nsteps, B, 2 * H), np.float32)
    for c in range(NCORES):
        d, qtr = divmod(c, 4)
        s = slice(NB * qtr, NB * (qtr + 1))
        o = res.results[c]["out"].astype(np.float32)
        o = o.transpose(1, 3, 2, 0).reshape(nsteps, NB, 512)
        if d == 1:
            o = o[::-1]
        out[:, s, 512 * d:512 * (d + 1)] = o
    kernel._last_results = res
    return out


# revision 3
# speedup vs baseline: 1.0358x; 1.0037x over previous
"""Bidirectional LSTM on 8 NeuronCores — v7: per-gate-block PSUM tiles.

Structure (dir-sharding, flipped orientation, transpose-gather, -1e9 mask)
as v2/v4, but gpre never leaves PSUM:
  - The gates PSUM is organized in 8-step groups: one [128, 2048] f32 tile
    (4 banks) holds cols m*128 + 16*t8 + b for the group's 8 steps.
  - Phase-1 (Wih GEMM + mask + bias, all PE matmuls) writes each group's
    tile one group ahead of the recurrence; the per-step h-matmuls then
    accumulate on top (start=False) and the activations read the psum
    directly. No eye-inject, no psum->sbuf copies: Act/DVE run ONLY the
    recurrence chain, so the tile scheduler cannot wedge bulk work into
    the latency-critical path.
  - PSUM budget: 2 group tiles x 4 banks = all 8 banks.
"""

import sys

sys.path.insert(0, "/opt/trn_rl_repo")

import numpy as np
import ml_dtypes

L, B, E, V = 512, 64, 512, 32000
H = 512
NB = 16
NCORES = 8
GRP = 8            # steps per psum group
WIN = 32           # steps per output-ring window

_BUILT = {}


def _split_sync_waits(nc, max_waits=1):
    import concourse.mybir as mybir

    ctr = 0
    for fn in nc.m.functions:
        for blk in fn.blocks:
            out = []
            changed = False
            for inst in blk.instructions:
                si = inst.sync_info
                if si is not None and si.on_wait and len(si.on_wait) > max_waits:
                    waits = list(si.on_wait)
                    extra, keep = waits[:-max_waits], waits[-max_waits:]
                    for i in range(0, len(extra), max_waits):
                        ctr += 1
                        nop = mybir.InstNoOp(
                            name=f"bass_waitsplit_{ctr}", ins=[], outs=[])
                        nop.engine = inst.engine
                        nop.sync_info = mybir.SyncInfo(
                            on_wait=extra[i:i + max_waits], on_update=[])
                        out.append(nop)
                    si.on_wait = keep
                    changed = True
                out.append(inst)
            if changed:
                blk.instructions[:] = out


def _build(nsteps=L):
    key = nsteps
    if key in _BUILT:
        return _BUILT[key]
    import concourse.bass as bass
    import concourse.mybir as mybir
    import concourse.tile as tile
    from concourse.masks import make_identity

    f32 = mybir.dt.float32
    bf16 = mybir.dt.bfloat16
    i16 = mybir.dt.int16
    AF = mybir.ActivationFunctionType
    ngrp = nsteps // GRP
    NGB = GRP * NB           # idx count per group (128)

    nc = bass.Bass()
    emb = nc.dram_tensor("emb", [V, E], bf16, kind="ExternalInput")
    toks = nc.dram_tensor("toks", [128, nsteps // GRP], mybir.dt.int32,
                          kind="ExternalInput")
    maskbar_d = nc.dram_tensor("maskbar", [1, NB * nsteps], bf16,
                               kind="ExternalInput")
    wihT_d = nc.dram_tensor("wihT", [4, 128, 2048], bf16, kind="ExternalInput")
    whhT_d = nc.dram_tensor("whhT", [4, 128, 2048], bf16, kind="ExternalInput")
    gbiasT_d = nc.dram_tensor("gbiasT", [1, 2048], bf16, kind="ExternalInput")
    out_d = nc.dram_tensor("out", [128, nsteps, 4, NB], bf16,
                           kind="ExternalOutput")

    with tile.TileContext(nc) as tc:
        with (
            tc.tile_pool(name="persist", bufs=1) as pp,
            tc.tile_pool(name="xT", bufs=3) as xp,
            tc.tile_pool(name="xr", bufs=3) as xrp,
            tc.tile_pool(name="ring", bufs=2) as rp,
            tc.tile_pool(name="ew", bufs=3) as ep,
            tc.tile_pool(name="cst", bufs=2) as cp,
            tc.tile_pool(name="gps", bufs=2, space="PSUM") as gpsp,
        ):
            # ---- persistent SBUF ----
            wihT = pp.tile([128, 4, 2048], bf16)
            whhT = pp.tile([128, 4, 2048], bf16)
            for k in range(4):
                nc.sync.dma_start(wihT[:, k, :], wihT_d[k])
                nc.sync.dma_start(whhT[:, k, :], whhT_d[k])
            # bias as a K=1 stationary: gbiasT[0, j] = bias of gate j
            gbiasT = pp.tile([1, 2048], bf16)
            nc.sync.dma_start(gbiasT[:], gbiasT_d[:])
            toks_t = pp.tile([128, nsteps // GRP], mybir.dt.int32)
            nc.sync.dma_start(toks_t[:], toks[:])
            maskbar = pp.tile([1, NB * nsteps], bf16)
            nc.sync.dma_start(maskbar[:], maskbar_d[:])
            mneg = pp.tile([1, 128], bf16)
            nc.vector.memset(mneg[:], -1e9)
            ones1 = pp.tile([1, 128], bf16)
            nc.vector.memset(ones1[:], 1.0)
            identb = pp.tile([128, 128], bf16)
            make_identity(nc, identb[:])

            def gather(g, tiles):
                """Gather group g's embeddings and transpose to xT via the
                PE, borrowing the group's own (still idle) gate psum tiles
                as transpose scratch."""
                xr = xrp.tile([128, E], bf16)
                nc.gpsimd.indirect_dma_start(
                    out=xr[:], out_offset=None, in_=emb[:],
                    in_offset=bass.IndirectOffsetOnAxis(
                        ap=toks_t[:, g:g + 1], axis=0))
                xT = xp.tile([128, 4, NGB], bf16)
                scr = [tiles[0][:, 0:64], tiles[0][:, 512:576],
                       tiles[1][:, 0:64], tiles[2][:, 0:64]]
                for k in range(4):
                    scratch = scr[k].bitcast(bf16)
                    nc.tensor.transpose(out=scratch, in_=xr[:, 128 * k:
                                                            128 * (k + 1)],
                                        identity=identb[:])
                    nc.vector.tensor_copy(xT[:, k, :], scratch)
                return xT

            def p1_group(xT_g, tiles, g):
                """gpre for group g -> psum tiles [fi (2 banks), o, g]:
                col = 128*m_local + 16*t8 + b."""
                for ti, ms in ((0, range(0, 8)), (1, range(8, 12)),
                               (2, range(12, 16))):
                    ps = tiles[ti]
                    for mp, m in enumerate(ms):
                        o = 128 * mp
                        for k in range(4):
                            # start=True zeroes a whole 2KB psum bank; only
                            # the first chunk of each bank may set it
                            nc.tensor.matmul(
                                ps[:, o:o + 128],
                                wihT[:, k, 128 * m:128 * (m + 1)],
                                xT_g[:, k, :],
                                start=(k == 0 and mp % 4 == 0), stop=False,
                                skip_group_check=True)
                        if m < 8:
                            nc.tensor.matmul(
                                ps[:, o:o + 128], mneg[:, :],
                                maskbar[:, NGB * g:NGB * (g + 1)],
                                start=False, stop=False,
                                skip_group_check=True)
                        nc.tensor.matmul(
                            ps[:, o:o + 128],
                            gbiasT[:, 128 * m:128 * (m + 1)],
                            ones1[:, 0:NGB],
                            start=False, stop=True,
                            skip_group_check=True)

            def alloc_group():
                fi = gpsp.tile([128, 1024], f32, space="PSUM", name="gps_fi")
                o_ = gpsp.tile([128, 512], f32, space="PSUM", name="gps_o")
                g_ = gpsp.tile([128, 512], f32, space="PSUM", name="gps_g")
                return [fi, o_, g_]

            # group 0 phase 1 up front
            gps_cur = alloc_group()
            xT0 = gather(0, gps_cur)
            p1_group(xT0, gps_cur, 0)
            gps_next = None

            ring_prev = None
            ring_cur = None
            c_prev = None

            for t in range(nsteps):
                g, t8 = t // GRP, t % GRP
                w, tp = t // WIN, t % WIN
                if t8 == 0 and g + 1 < ngrp:
                    with tc.high_priority(offset=-1000000):
                        gps_next = alloc_group()
                        xT_g = gather(g + 1, gps_next)
                        p1_group(xT_g, gps_next, g + 1)
                if tp == 0:
                    ring_prev, ring_cur = ring_cur, rp.tile(
                        [128, WIN, 4 * NB], bf16)

                # ---- recurrence matmuls: accumulate h @ WhhT into psum ----
                co = 16 * t8
                if t > 0:
                    hmov = (ring_prev[:, WIN - 1, :] if tp == 0
                            else ring_cur[:, tp - 1, :])
                    for m in range(16):
                        ti, mp = ((0, m) if m < 8 else
                                  (1, m - 8) if m < 12 else (2, m - 12))
                        for k in range(4):
                            nc.tensor.matmul(
                                gps_cur[ti][:, 128 * mp + co:
                                            128 * mp + co + 16],
                                whhT[:, k, 128 * m:128 * (m + 1)],
                                hmov[:, NB * k:NB * (k + 1)],
                                start=False, stop=(k == 3),
                                skip_group_check=True)

                # ---- elementwise chain ----
                q = 4 * NB  # 64
                # per-gate psum tiles give each activation a precise PE
                # wait (its own block's last matmul); Act same-engine pins
                # are stripped post-build
                def gv(ti):
                    return gps_cur[ti][:].rearrange(
                        "p (m c) -> p m c", c=128)[:, :, co:co + 16]
                sig = ep.tile([128, 2 * q], f32)
                nc.scalar.activation(sig[:], gv(0), AF.Sigmoid)
                tg = ep.tile([128, q], f32)
                nc.scalar.activation(tg[:], gv(2), AF.Tanh)
                sigo = ep.tile([128, q], f32)
                nc.scalar.activation(sigo[:], gv(1), AF.Sigmoid)

                c_new = cp.tile([128, q], f32, tag="c_state")
                if c_prev is None:
                    nc.vector.tensor_mul(c_new[:], sig[:, q:2 * q], tg[:])
                else:
                    t2 = ep.tile([128, q], f32)
                    nc.vector.tensor_mul(t2[:], sig[:, 0:q], c_prev[:])
                    t1 = ep.tile([128, q], f32)
                    nc.vector.tensor_mul(t1[:], sig[:, q:2 * q], tg[:])
                    nc.vector.tensor_add(c_new[:], t1[:], t2[:])
                tc_ = ep.tile([128, q], f32)
                nc.scalar.activation(tc_[:], c_new[:], AF.Tanh)
                nc.vector.tensor_mul(ring_cur[:, tp, :], sigo[:], tc_[:])
                c_prev = c_new

                if tp == WIN - 1:
                    nc.sync.dma_start(
                        out_d[:, WIN * w:WIN * (w + 1)], ring_cur[:])
                if t8 == GRP - 1:
                    gps_cur = gps_next

    _BUILT[key] = nc
    return nc


def _strip_act_pins(nc):
    import concourse.mybir as mybir
    n = 0
    for fn in nc.m.functions:
        for blk in fn.blocks:
            for inst in blk.instructions:
                if type(inst).__name__ != 'InstActivation':
                    continue
                si = inst.sync_info
                if si is None or not si.on_wait:
                    continue
                ins0 = inst.ins[0]
                mref = getattr(ins0, 'memref', '')
                if not (isinstance(mref, str) and mref.startswith('gps')):
                    continue
                keep = [w for w in si.on_wait
                        if not getattr(w, 'ant_name', '')
                        .startswith('Activation')]
                if len(keep) != len(si.on_wait):
                    si.on_wait = keep
                    n += 1
    return n


NOSPLIT = False


def _ensure_split(nc):
    if NOSPLIT:
        return
    if not getattr(nc, "_waitsplit_done", False):
        _strip_act_pins(nc)
        _split_sync_waits(nc)
        nc._waitsplit_done = True


# gate row order [f | i | o | g] (reference order is i, f, g, o)
_PERM = np.concatenate([
    np.arange(512, 1024), np.arange(0, 512),
    np.arange(1536, 2048), np.arange(1024, 1536)])


def _prep_weights(W_ih, W_hh, b_ih, b_hh):
    wihT = np.ascontiguousarray(
        np.asarray(W_ih, np.float32)[_PERM].T.reshape(4, 128, 2048)
    ).astype(ml_dtypes.bfloat16)
    whhT = np.ascontiguousarray(
        np.asarray(W_hh, np.float32)[_PERM].T.reshape(4, 128, 2048)
    ).astype(ml_dtypes.bfloat16)
    gbiasT = np.ascontiguousarray(
        (np.asarray(b_ih, np.float32)
         + np.asarray(b_hh, np.float32))[_PERM].reshape(1, 2048)
    ).astype(ml_dtypes.bfloat16)
    return wihT, whhT, gbiasT


def kernel(tokens, mask, emb_table, W_ih_f, W_hh_f, b_ih_f, b_hh_f,
           W_ih_b, W_hh_b, b_ih_b, b_hh_b, _nsteps=L, _trace=False):
    from concourse.bass_utils import run_bass_kernel_spmd

    tokens = np.asarray(tokens)
    mask = np.asarray(mask, dtype=np.float32)
    emb_bf = np.ascontiguousarray(
        np.asarray(emb_table, dtype=np.float32)).astype(ml_dtypes.bfloat16)

    wf = _prep_weights(W_ih_f, W_hh_f, b_ih_f, b_hh_f)
    wb = _prep_weights(W_ih_b, W_hh_b, b_ih_b, b_hh_b)

    nsteps = _nsteps
    nc = _build(nsteps)
    _ensure_split(nc)

    in_maps = []
    for c in range(NCORES):
        d, qtr = divmod(c, 4)
        s = slice(NB * qtr, NB * (qtr + 1))
        tok_c = tokens[:nsteps, s]
        mask_c = mask[:nsteps, s]
        if d == 1:
            tok_c, mask_c = tok_c[::-1], mask_c[::-1]
        tc_clip = np.clip(tok_c, 0, V - 1).astype(np.int32)
        # row p of gather group g holds token (t = GRP*g + p//16, b = p%16)
        toks_sb = np.ascontiguousarray(
            tc_clip.reshape(nsteps // GRP, GRP * NB).T)
        maskbar = np.ascontiguousarray(
            (1.0 - mask_c).reshape(1, NB * nsteps)).astype(ml_dtypes.bfloat16)
        wihT, whhT, gbiasT = wf if d == 0 else wb
        in_maps.append({
            "emb": emb_bf,
            "toks": toks_sb,
            "maskbar": maskbar,
            "wihT": wihT,
            "whhT": whhT,
            "gbiasT": gbiasT,
        })

    res = run_bass_kernel_spmd(nc, in_maps, core_ids=list(range(NCORES)),
                               trace=_trace)
    out = np.empty((nsteps, B, 2 * H), np.float32)
    for c in range(NCORES):
        d, qtr = divmod(c, 4)
        s = slice(NB * qtr, NB * (qtr + 1))
        o = res.results[c]["out"].astype(np.float32)
        o = o.transpose(1, 3, 2, 0).reshape(nsteps, NB, 512)
        if d == 1:
            o = o[::-1]
        out[:, s, 512 * d:512 * (d + 1)] = o
    kernel._last_results = res
    return out
